# revision 8
# baseline (speedup 1.0000x reference)
"""DMPNN message-passing kernel for 8 Trainium2 NeuronCores.

Strategy (hardcoded for E=8192, N=4096, H=256, T=4):
  - Shard edges across the 8 cores (1024 rows of adj_ee each); shard nodes
    for the aggregation stage (512 rows of adj_ne each).
  - Adjacency shards ship pre-transposed in fp8e4 (exact for a 0/1
    adjacency) and stay RESIDENT in SBUF across all 4 iterations (8 MB),
    removing 24 MB of repeated HBM traffic per core.
  - h is quantized to fp8e4 before each AllGather; the dominant
    adjacency matmuls run fp8 x fp8 with perf_mode=DoubleRow (256-deep
    contraction per instruction, 2x bf16 rate on HW).
  - Each iteration is split into two 512-edge phases; each phase LN+updates
    its edges and launches its half of the AllGather while the other
    phase's matmuls keep the PE busy. The next iteration consumes gathered
    k-chunks phase-0-first so the second collective hides behind compute.
  - LayerNorm uses the transposed-stats trick: (1/H)-matmuls against m.T
    give mean / E[m^2] replicated over partitions; LN gain is folded into
    W_pass (host-side), the affine is fused into bf16 ops feeding the
    bf16 update matmul.
  - Final stage: node aggregation (adj_ne, fp8 DoubleRow), LN, dense+relu,
    LN, per-core column sum; host sums the 8 per-core partials.

Numerics: fp8 is exact for the adjacency; h in fp8e4 measures ~1e-2 rel
err end-to-end (gate 2e-2) in a bit-accurate numpy model; everything
after the adjacency matmul stays fp32/bf16.
"""

import numpy as np
import ml_dtypes

E, N, NODE_D, EDGE_D, H, T = 8192, 4096, 64, 16, 256, 4
NCORES = 8
EC = E // NCORES       # 1024 edges per core
NNC = N // NCORES      # 512 nodes per core
KK = E // 256          # 32 double-row contraction chunks
ES = EC // 128         # 8 edge subtiles per core
NS = NNC // 128        # 4 node subtiles per core
NPH = 2                # phases per iteration (512 edges each)
EPH = EC // NPH        # 512 edges per phase
EPS = 1e-6
BF = ml_dtypes.bfloat16
F8 = ml_dtypes.float8_e4m3

LAST_RESULT = None     # BassKernelResults of the most recent run (for test.py)

_prog_cache = {}

# how many kk-chunks of the next phase's big matmul to emit between a
# phase's last matmul and its stats / update work (hides DVE/ACT latency
# without delaying the phase's AllGather launch too much)
NLEAD_STATS = 6
NLEAD_UPD = 8


def _build(flags, repeat=1, nocc=False):
    import concourse.bacc as bacc
    import concourse.mybir as mybir
    import concourse.tile as tile

    f32 = mybir.dt.float32
    bf16 = mybir.dt.bfloat16
    f8 = mybir.dt.float8e4
    rg = [list(range(NCORES))]

    nc = bacc.Bacc("TRN2", target_bir_lowering=False, debug=False,
                   num_devices=NCORES)

    aT = nc.declare_dram_parameter("aT", [E, EC], f8, isOutput=False)
    aneT = nc.declare_dram_parameter("aneT", [E, NNC], f8, isOutput=False)
    XcT = nc.declare_dram_parameter("XcT", [128, EC], bf16, isOutput=False)
    Wi = nc.declare_dram_parameter("Wi", [128, H], bf16, isOutput=False)
    W2s = nc.declare_dram_parameter("W2s", [T, H, H], bf16, isOutput=False)
    nfT = nc.declare_dram_parameter("nfT", [128, NNC], bf16, isOutput=False)
    Wn = nc.declare_dram_parameter("Wn", [128, H], bf16, isOutput=False)
    Wm = nc.declare_dram_parameter("Wm", [H, H], bf16, isOutput=False)
    c1 = nc.declare_dram_parameter("c1", [1, H], f32, isOutput=False)
    c2s = nc.declare_dram_parameter("c2s", [T, H], f32, isOutput=False)
    c3 = nc.declare_dram_parameter("c3", [1, H], f32, isOutput=False)
    gagg = nc.declare_dram_parameter("gagg", [1, H], f32, isOutput=False)
    bagg = nc.declare_dram_parameter("bagg", [1, H], f32, isOutput=False)
    out = nc.declare_dram_parameter("out", [1, H], f32, isOutput=True)

    ag_in = [[nc.dram_tensor(f"agin{t}_{ph}", [EPH, H], f8)
              for ph in range(NPH)] for t in range(T + 1)]
    ag_out = [[nc.dram_tensor(f"agout{t}_{ph}", [E // NPH, H], f8,
                              addr_space="Shared")
               for ph in range(NPH)] for t in range(T + 1)]

    with tile.TileContext(nc) as tc:
        with (
            tc.tile_pool(name="singles", bufs=1) as singles,
            tc.tile_pool(name="hb", bufs=2) as hbpool,
            tc.tile_pool(name="hsh", bufs=2) as hshpool,
            tc.tile_pool(name="work", bufs=2) as work,
            # PSUM budget (8 banks of 2KB/partition, bank-granular):
            #   psm   4 x [128,512]f32 (A+B phase accumulators)
            #   psst  2 x [128,512]f32 (stats + small accumulators, rotating)
            #   psout 1 x [1,256]      (column-sum accumulator)
            tc.tile_pool(name="psm", bufs=4, space="PSUM") as psmpool,
            tc.tile_pool(name="psst", bufs=2, space="PSUM") as psstpool,
            tc.tile_pool(name="psout", bufs=1, space="PSUM") as psoutpool,
        ):
            # ---- static tiles ----
            xct_sb = singles.tile([128, EC], bf16)
            nc.sync.dma_start(xct_sb[:], XcT[:, :])
            wi_sb = singles.tile([128, H], bf16)
            nc.sync.dma_start(wi_sb[:], Wi[:, :])
            w2_sb = singles.tile([128, T, 2, H], bf16)
            nc.sync.dma_start(
                w2_sb[:], W2s.ap().rearrange("t (kk p) n -> p t kk n", p=128))
            nft_sb = singles.tile([128, NNC], bf16)
            nc.sync.dma_start(nft_sb[:], nfT[:, :])
            wn_sb = singles.tile([128, H], bf16)
            nc.sync.dma_start(wn_sb[:], Wn[:, :])
            wm_sb = singles.tile([128, 2, H], bf16)
            nc.sync.dma_start(
                wm_sb[:], Wm.ap().rearrange("(kk p) n -> p kk n", p=128))

            # adjacency resident in SBUF (8 MB + 4 MB), k-chunked loads so
            # the first matmuls can start early
            aT_sb = singles.tile([128, KK, 2, EC], f8)
            aTr = aT.ap().rearrange("(kk two p) c -> p kk two c", two=2, p=128)
            for c in range(4):
                ksl = slice(c * (KK // 4), (c + 1) * (KK // 4))
                nc.sync.dma_start(aT_sb[:, ksl], aTr[:, ksl])
            ane_sb = singles.tile([128, KK, 2, NNC], f8)
            aner = aneT.ap().rearrange("(kk two p) c -> p kk two c",
                                       two=2, p=128)
            for c in range(2):
                ksl = slice(c * (KK // 2), (c + 1) * (KK // 2))
                nc.sync.dma_start(ane_sb[:, ksl], aner[:, ksl])

            onesH = singles.tile([128, 128], bf16)
            nc.vector.memset(onesH[:], 1.0 / H)
            onescol = singles.tile([128, 1], bf16)
            nc.vector.memset(onescol[:], 1.0)
            eps_sb = singles.tile([128, 1], f32)
            nc.vector.memset(eps_sb[:], EPS)
            h0_sb = singles.tile([128, ES, H], f32)

            def bcast_load(src_ap):
                t_ = singles.tile([128, H], f32)
                nc.sync.dma_start(t_[:], src_ap.to_broadcast([128, H]))
                return t_

            c1_bc = bcast_load(c1.ap()) if flags["c1"] else None
            c2_bc = [bcast_load(c2s.ap()[t_i:t_i + 1, :]) if flags["c2"][t_i]
                     else None for t_i in range(T)]
            c3_bc = bcast_load(c3.ap()) if flags["c3"] else None
            gagg_bc = bcast_load(gagg.ap()) if flags["gagg"] else None
            bagg_bc = bcast_load(bagg.ap()) if flags["bagg"] else None

            for _rep in range(repeat):
                _pipeline(nc, tile, mybir, _rep, nocc, rg,
                          pools=(singles, hbpool, hshpool, work, psmpool,
                                 psstpool, psoutpool),
                          tens=(out, ag_in, ag_out),
                          sbufs=(xct_sb, wi_sb, w2_sb, nft_sb, wn_sb, wm_sb,
                                 aT_sb, ane_sb, onesH, onescol, eps_sb, h0_sb),
                          bcs=(c1_bc, c2_bc, c3_bc, gagg_bc, bagg_bc))

    nc.compile()
    return nc


def _pipeline(nc, tile, mybir, _rep, nocc, rg, pools, tens, sbufs, bcs):
    f32 = mybir.dt.float32
    bf16 = mybir.dt.bfloat16
    f8 = mybir.dt.float8e4
    AF = mybir.ActivationFunctionType
    DR = mybir.MatmulPerfMode.DoubleRow
    (singles, hbpool, hshpool, work, psmpool, psstpool,
     psoutpool) = pools
    (out, ag_in, ag_out) = tens
    (xct_sb, wi_sb, w2_sb, nft_sb, wn_sb, wm_sb,
     aT_sb, ane_sb, onesH, onescol, eps_sb, h0_sb) = sbufs
    (c1_bc, c2_bc, c3_bc, gagg_bc, bagg_bc) = bcs

    # ---- gather plumbing -------------------------------------------------
    def launch_gather(t_idx, ph, hsh_tile):
        """hsh_tile [128, ES//NPH, H] fp8 -> ag_in -> AllGather."""
        nc.sync.dma_start(
            ag_in[t_idx][ph].ap().rearrange("(es p) h -> p es h", p=128),
            hsh_tile[:])
        if not nocc:
            nc.gpsimd.collective_compute(
                "AllGather", mybir.AluOpType.bypass, replica_groups=rg,
                ins=[ag_in[t_idx][ph].ap().opt()],
                outs=[ag_out[t_idx][ph].ap().opt()])

    def load_hb(t_idx, ph):
        """Load gathered h for phase ph of stage t into 2 SBUF chunks.

        Chunk c holds ranks g in [4c, 4c+4); layout [128, g4, l, two, H],
        where (g,l) maps to global kk = 4g + 2*ph + l.
        """
        chunks = []
        for c in range(2):
            hg = hbpool.tile([128, 4, 2, 2, H], f8, tag=f"hb{ph}{c}",
                             name=f"hb_{_rep}_{t_idx}_{ph}_{c}")
            if nocc:
                # timeline-sim variant (no collective support): emulate the
                # gather's local DMA traffic by reading the shard 8x.
                src = ag_in[t_idx][ph].ap().rearrange(
                    "(l two p) h -> p l two h", two=2, p=128)
                for g4 in range(4):
                    nc.sync.dma_start(hg[:, g4], src)
            else:
                src = ag_out[t_idx][ph].ap().rearrange(
                    "(g l two p) h -> p g l two h", l=2, two=2, p=128)
                nc.sync.dma_start(hg[:], src[:, c * 4:(c + 1) * 4])
            chunks.append(hg)
        return chunks

    def hb_slice(hb, kk, half):
        """Stationary [128, 2, 128] fp8 for global kk-chunk, given hb as
        {ph: [chunk0, chunk1]} of the current stage."""
        g, r = divmod(kk, 4)
        ph, l = divmod(r, 2)
        return hb[ph][g // 4][:, g % 4, l, :, half * 128:(half + 1) * 128]

    # kk consumption order: phase-0-delivered chunks first
    KK_ORDER = [4 * g + 2 * ph + l
                for ph in range(NPH) for g in range(NCORES) for l in range(2)]

    # ---- h0 = relu(X @ W_init + b_init); quantize + gather ---------------
    hb = {}
    hsh0 = {ph: hshpool.tile([128, ES // NPH, H], f8, tag=f"hsh{ph}",
                             name=f"hsh_{_rep}_init_{ph}")
            for ph in range(NPH)}
    for ph in range(NPH):
        for e4 in range(ES // NPH):
            es = ph * (ES // NPH) + e4
            ps = psstpool.tile([128, H], f32, tag="psst",
                               name=f"psi_{_rep}_{es}")
            nc.tensor.matmul(ps[:], lhsT=xct_sb[:, es * 128:(es + 1) * 128],
                             rhs=wi_sb[:], start=True, stop=True)
            if c1_bc is not None:
                tmp = work.tile([128, H], f32, tag="tmp")
                nc.vector.tensor_add(tmp[:], ps[:], c1_bc[:])
                nc.scalar.activation(h0_sb[:, es], tmp[:], AF.Relu)
                nc.scalar.activation(hsh0[ph][:, e4], tmp[:], AF.Relu)
            else:
                nc.scalar.activation(h0_sb[:, es], ps[:], AF.Relu)
                nc.scalar.activation(hsh0[ph][:, e4], ps[:], AF.Relu)
        launch_gather(0, ph, hsh0[ph])
    for ph in range(NPH):
        hb[ph] = load_hb(0, ph)

    # ---- T message-passing iterations ------------------------------------
    def emit_mm(t, ps_m, et, lo, hi, hb_cur):
        for kk in KK_ORDER[lo:hi]:
            for half in range(2):
                nc.tensor.matmul(
                    ps_m[half][:], lhsT=hb_slice(hb_cur, kk, half),
                    rhs=aT_sb[:, kk, :, et * EPH:(et + 1) * EPH],
                    start=(kk == KK_ORDER[0]), stop=(kk == KK_ORDER[-1]),
                    perf_mode=DR)

    def ln_stats(ps_m, width, name):
        """From psum m.T halves, compute mT (bf16), psmean, pssq."""
        mT = work.tile([128, 2, width], bf16, tag=f"mT{width}",
                       name=f"mT_{name}")
        sq = work.tile([128, 2, width], bf16, tag=f"sq{width}",
                       name=f"sq_{name}")
        for half in range(2):
            nc.vector.tensor_copy(mT[:, half], ps_m[half][:])
            nc.scalar.activation(sq[:, half], ps_m[half][:], AF.Square)
        return mT, sq

    def ln_matmuls(mT, sq, width, name):
        psmean = psstpool.tile([128, width], f32, tag="psst",
                               name=f"psmean_{name}")
        nc.tensor.matmul(psmean[:], lhsT=onesH[:], rhs=mT[:, 0],
                         start=True, stop=False)
        nc.tensor.matmul(psmean[:], lhsT=onesH[:], rhs=mT[:, 1],
                         start=False, stop=True)
        pssq = psstpool.tile([128, width], f32, tag="psst",
                             name=f"pssq_{name}")
        nc.tensor.matmul(pssq[:], lhsT=onesH[:], rhs=sq[:, 0],
                         start=True, stop=False)
        nc.tensor.matmul(pssq[:], lhsT=onesH[:], rhs=sq[:, 1],
                         start=False, stop=True)
        return psmean, pssq

    def ln_finish(mT, psmean, pssq, width, name):
        """cln = (mT - mean) * rsqrt(var + eps), bf16."""
        msq = work.tile([128, width], f32, tag=f"msq{width}",
                        name=f"msq_{name}")
        nc.scalar.activation(msq[:], psmean[:], AF.Square)
        var = work.tile([128, width], f32, tag=f"var{width}",
                        name=f"var_{name}")
        nc.vector.tensor_sub(var[:], pssq[:], msq[:])
        srt = work.tile([128, width], f32, tag=f"srt{width}",
                        name=f"srt_{name}")
        nc.scalar.activation(srt[:], var[:], AF.Sqrt, bias=eps_sb[:],
                             scale=1.0)
        rstd = work.tile([128, width], bf16, tag=f"rstd{width}",
                         name=f"rstd_{name}")
        with nc.allow_low_precision(reason="bf16 LN rstd, within err budget"):
            nc.vector.reciprocal(out=rstd[:], in_=srt[:])
        cln = work.tile([128, 2, width], bf16, tag=f"cln{width}",
                        name=f"cln_{name}")
        for half in range(2):
            nc.vector.tensor_sub(cln[:, half], mT[:, half], psmean[:])
            nc.vector.tensor_mul(cln[:, half], cln[:, half], rstd[:])
        return cln

    def update_phase(t, ph, cln, hsh_tile):
        """h = relu(h0 + cln @ W2[t]) for the 4 es-blocks of phase ph,
        write fp8 into hsh_tile, then gather."""
        for e4 in range(ES // NPH):
            es = ph * (ES // NPH) + e4
            lsl = slice(e4 * 128, (e4 + 1) * 128)
            psu = psstpool.tile([128, H], f32, tag="psst",
                               name=f"psu_{_rep}_{t}_{es}")
            nc.tensor.matmul(psu[:], lhsT=cln[:, 0, lsl], rhs=w2_sb[:, t, 0],
                             start=True, stop=False)
            nc.tensor.matmul(psu[:], lhsT=cln[:, 1, lsl], rhs=w2_sb[:, t, 1],
                             start=False, stop=True)
            tmp = work.tile([128, H], f32, tag="tmp",
                            name=f"upd_{_rep}_{t}_{es}")
            nc.vector.tensor_add(tmp[:], psu[:], h0_sb[:, es])
            if c2_bc[t] is not None:
                nc.vector.tensor_add(tmp[:], tmp[:], c2_bc[t][:])
            nc.scalar.activation(hsh_tile[:, e4], tmp[:], AF.Relu)
        launch_gather(t + 1, ph, hsh_tile)

    for t in range(T):
        ps_m = {et: [psmpool.tile([128, EPH], f32, tag="psm",
                                  name=f"psm_{_rep}_{t}_{et}_{h_}")
                     for h_ in range(2)] for et in range(NPH)}
        hsh = {ph: hshpool.tile([128, ES // NPH, H], f8, tag=f"hsh{ph}",
                                name=f"hsh_{_rep}_{t}_{ph}")
               for ph in range(NPH)}
        # phase A matmuls
        emit_mm(t, ps_m[0], 0, 0, KK, hb)
        # leading B matmuls hide A's PSUM->SBUF copy latency
        emit_mm(t, ps_m[1], 1, 0, NLEAD_STATS, hb)
        mT_a, sq_a = ln_stats(ps_m[0], EPH, f"{_rep}_{t}_0")
        psmean_a, pssq_a = ln_matmuls(mT_a, sq_a, EPH, f"{_rep}_{t}_0")
        emit_mm(t, ps_m[1], 1, NLEAD_STATS, NLEAD_STATS + NLEAD_UPD, hb)
        cln_a = ln_finish(mT_a, psmean_a, pssq_a, EPH, f"{_rep}_{t}_0")
        update_phase(t, 0, cln_a, hsh[0])
        # rest of phase B matmuls
        emit_mm(t, ps_m[1], 1, NLEAD_STATS + NLEAD_UPD, KK, hb)
        mT_b, sq_b = ln_stats(ps_m[1], EPH, f"{_rep}_{t}_1")
        psmean_b, pssq_b = ln_matmuls(mT_b, sq_b, EPH, f"{_rep}_{t}_1")
        cln_b = ln_finish(mT_b, psmean_b, pssq_b, EPH, f"{_rep}_{t}_1")
        update_phase(t, 1, cln_b, hsh[1])
        hb = {}
        for ph in range(NPH):
            hb[ph] = load_hb(t + 1, ph)

    # ---- node aggregation: m_v.T = h.T @ adj_ne_shard.T -------------------
    ps_mv = [psmpool.tile([128, NNC], f32, tag="psm",
                          name=f"psmv_{_rep}_{h_}")
             for h_ in range(2)]
    for kk in KK_ORDER:
        for half in range(2):
            nc.tensor.matmul(
                ps_mv[half][:], lhsT=hb_slice(hb, kk, half),
                rhs=ane_sb[:, kk], start=(kk == KK_ORDER[0]),
                stop=(kk == KK_ORDER[-1]), perf_mode=DR)

    mT_v, sq_v = ln_stats(ps_mv, NNC, f"{_rep}_v")
    psmean_v, pssq_v = ln_matmuls(mT_v, sq_v, NNC, f"{_rep}_v")
    cln_v = ln_finish(mT_v, psmean_v, pssq_v, NNC, f"{_rep}_v")

    # ---- h_v = relu(nf @ Wagg[:64] + m_v_ln @ Wagg[64:] + c3);
    #      LN again; column-sum over nodes ----
    ps_out = psoutpool.tile([1, H], f32, tag="psout")
    for ns in range(NS):
        sl = slice(ns * 128, (ns + 1) * 128)
        ps_hv = psstpool.tile([128, H], f32, tag="psst",
                                name=f"pshv_{_rep}_{ns}")
        nc.tensor.matmul(ps_hv[:], lhsT=nft_sb[:, sl], rhs=wn_sb[:],
                         start=True, stop=False)
        nc.tensor.matmul(ps_hv[:], lhsT=cln_v[:, 0, sl],
                         rhs=wm_sb[:, 0], start=False, stop=False)
        nc.tensor.matmul(ps_hv[:], lhsT=cln_v[:, 1, sl],
                         rhs=wm_sb[:, 1], start=False, stop=True)
        hv = work.tile([128, H], f32, tag="hv", name=f"hv_{_rep}_{ns}")
        if c3_bc is not None:
            nc.vector.tensor_add(hv[:], ps_hv[:], c3_bc[:])
            nc.vector.tensor_scalar_max(hv[:], hv[:], 0.0)
        else:
            nc.scalar.activation(hv[:], ps_hv[:], AF.Relu)
        stats = work.tile([128, 6], f32, tag="stats")
        nc.vector.bn_stats(out=stats[:], in_=hv[:])
        mv2 = work.tile([128, 2], f32, tag="mv2")
        nc.vector.bn_aggr(out=mv2[:], in_=stats[:])
        rstd2 = work.tile([128, 1], f32, tag="rstd2")
        nc.scalar.activation(rstd2[:], mv2[:, 1:2], AF.Sqrt,
                             bias=eps_sb[:], scale=1.0)
        nc.vector.reciprocal(out=rstd2[:], in_=rstd2[:])
        ln2 = work.tile([128, H], bf16, tag="ln2", name=f"ln2_{_rep}_{ns}")
        nc.vector.tensor_scalar(
            out=ln2[:], in0=hv[:], scalar1=mv2[:, 0:1],
            scalar2=rstd2[:], op0=mybir.AluOpType.subtract,
            op1=mybir.AluOpType.mult)
        if gagg_bc is not None:
            nc.vector.tensor_mul(ln2[:], ln2[:], gagg_bc[:])
        if bagg_bc is not None:
            nc.vector.tensor_add(ln2[:], ln2[:], bagg_bc[:])
        nc.tensor.matmul(ps_out[:], lhsT=onescol[:], rhs=ln2[:],
                         start=(ns == 0), stop=(ns == NS - 1))

    out_sb = work.tile([1, H], f32, tag="osb")
    nc.vector.tensor_copy(out_sb[:], ps_out[:])
    nc.sync.dma_start(out[:, :], out_sb[:])


def prepare(inputs, repeat=1, nocc=False):
    """Host-side prep: returns (nc, in_maps) for run_bass_kernel_spmd."""
    f = {k: np.ascontiguousarray(np.asarray(v), dtype=np.float32)
         for k, v in inputs.items()}

    X = np.concatenate(
        [f["edge_aligned_node_features"], f["dir_edge_features"]], axis=1)

    # Fold LN gains into the downstream weights (host-side, exact fp32).
    g_p, b_p = f["ln_pass_g"], f["ln_pass_b"]
    g_a, b_a = f["ln_agg_g"], f["ln_agg_b"]
    W2s = (g_p[:, None] * f["W_pass"]).astype(BF)                  # [T,H,H]
    c2s = (b_p @ f["W_pass"] + f["b_pass"]).astype(np.float32)     # [T,H]
    Wm = (g_a[:, None] * f["W_agg"][NODE_D:]).astype(BF)           # [H,H]
    c3 = (b_a @ f["W_agg"][NODE_D:] + f["b_agg"]).astype(np.float32)

    Wi = np.zeros((128, H), np.float32)
    Wi[:NODE_D + EDGE_D] = f["W_init"]
    Wn = np.zeros((128, H), np.float32)
    Wn[:NODE_D] = f["W_agg"][:NODE_D]

    flags = {
        "c1": bool(np.any(f["b_init"])),
        "c2": [bool(np.any(c2s[t])) for t in range(T)],
        "c3": bool(np.any(c3)),
        "gagg": not np.all(g_a == 1.0),
        "bagg": bool(np.any(b_a)),
    }
    key = (flags["c1"], tuple(flags["c2"]), flags["c3"], flags["gagg"],
           flags["bagg"], repeat, nocc)
    if key not in _prog_cache:
        _prog_cache[key] = _build(flags, repeat=repeat, nocc=nocc)
    nc = _prog_cache[key]

    shared = {
        "Wi": Wi.astype(BF),
        "W2s": W2s,
        "Wn": Wn.astype(BF),
        "Wm": Wm,
        "c1": f["b_init"].reshape(1, H),
        "c2s": c2s,
        "c3": c3.reshape(1, H),
        "gagg": g_a.reshape(1, H),
        "bagg": b_a.reshape(1, H),
    }
    in_maps = []
    for c in range(NCORES):
        er = slice(c * EC, (c + 1) * EC)
        nr = slice(c * NNC, (c + 1) * NNC)
        XcT = np.zeros((128, EC), np.float32)
        XcT[:NODE_D + EDGE_D] = X[er].T
        nfT = np.zeros((128, NNC), np.float32)
        nfT[:NODE_D] = f["node_features"][nr].T
        in_maps.append(dict(
            shared,
            aT=np.ascontiguousarray(f["adj_ee"][er].T).astype(F8),
            aneT=np.ascontiguousarray(f["adj_ne"][nr].T).astype(F8),
            XcT=XcT.astype(BF),
            nfT=nfT.astype(BF),
        ))
    return nc, in_maps


def kernel(**inputs) -> np.ndarray:
    global LAST_RESULT
    from concourse.bass_utils import run_bass_kernel_spmd

    nc, in_maps = prepare(inputs)
    LAST_RESULT = run_bass_kernel_spmd(nc, in_maps, list(range(NCORES)))
    parts = [LAST_RESULT.results[c]["out"] for c in range(NCORES)]
    return np.sum(parts, axis=0, dtype=np.float32).reshape(1, H)


# revision 10
# speedup vs baseline: 1.1577x; 1.1577x over previous
"""DMPNN message-passing kernel for 8 Trainium2 NeuronCores.

Strategy (hardcoded for E=8192, N=4096, H=256, T=4):
  - Shard edges across the 8 cores (1024 rows of adj_ee each); shard nodes
    for the aggregation stage (512 rows of adj_ne each).
  - Adjacency shards ship pre-transposed in fp8e4 (exact for a 0/1
    adjacency) and stay RESIDENT in SBUF across all 4 iterations (8 MB),
    removing 24 MB of repeated HBM traffic per core.
  - h is quantized to fp8e4 before each AllGather; the dominant
    adjacency matmuls run fp8 x fp8 with perf_mode=DoubleRow (256-deep
    contraction per instruction, 2x bf16 rate on HW).
  - Each iteration is split into two 512-edge phases; each phase LN+updates
    its edges and launches its half of the AllGather while the other
    phase's matmuls keep the PE busy. The next iteration consumes gathered
    k-chunks phase-0-first so the second collective hides behind compute.
  - LayerNorm uses the transposed-stats trick: (1/H)-matmuls against m.T
    give mean / E[m^2] replicated over partitions; LN gain is folded into
    W_pass (host-side), the affine is fused into bf16 ops feeding the
    bf16 update matmul.
  - Final stage: node aggregation (adj_ne, fp8 DoubleRow), LN, dense+relu,
    LN, per-core column sum; host sums the 8 per-core partials.

Numerics: fp8 is exact for the adjacency; h in fp8e4 measures ~1e-2 rel
err end-to-end (gate 2e-2) in a bit-accurate numpy model; everything
after the adjacency matmul stays fp32/bf16.
"""

import numpy as np
import ml_dtypes

E, N, NODE_D, EDGE_D, H, T = 8192, 4096, 64, 16, 256, 4
NCORES = 8
EC = E // NCORES       # 1024 edges per core
NNC = N // NCORES      # 512 nodes per core
KK = E // 256          # 32 double-row contraction chunks
ES = EC // 128         # 8 edge subtiles per core
NS = NNC // 128        # 4 node subtiles per core
NPH = 2                # phases per iteration (512 edges each)
EPH = EC // NPH        # 512 edges per phase
EPS = 1e-6
BF = ml_dtypes.bfloat16
F8 = ml_dtypes.float8_e4m3

LAST_RESULT = None     # BassKernelResults of the most recent run (for test.py)

_prog_cache = {}

# how many kk-chunks of the next phase's big matmul to emit between a
# phase's last matmul and its stats / update work (hides DVE/ACT latency
# without delaying the phase's AllGather launch too much)
NLEAD_STATS = 6
NLEAD_UPD = 8


def _build(flags, repeat=1, nocc=False):
    import concourse.bacc as bacc
    import concourse.mybir as mybir
    import concourse.tile as tile

    f32 = mybir.dt.float32
    bf16 = mybir.dt.bfloat16
    f8 = mybir.dt.float8e4
    rg = [list(range(NCORES))]

    nc = bacc.Bacc("TRN2", target_bir_lowering=False, debug=False,
                   num_devices=NCORES)

    aT = nc.declare_dram_parameter("aT", [E, EC], f8, isOutput=False)
    aneT = nc.declare_dram_parameter("aneT", [E, NNC], f8, isOutput=False)
    XcT = nc.declare_dram_parameter("XcT", [128, EC], bf16, isOutput=False)
    XfT = nc.declare_dram_parameter("XfT", [128, E], bf16, isOutput=False)
    Wi = nc.declare_dram_parameter("Wi", [128, H], bf16, isOutput=False)
    W2s = nc.declare_dram_parameter("W2s", [T, H, H], bf16, isOutput=False)
    nfT = nc.declare_dram_parameter("nfT", [128, NNC], bf16, isOutput=False)
    Wn = nc.declare_dram_parameter("Wn", [128, H], bf16, isOutput=False)
    Wm = nc.declare_dram_parameter("Wm", [H, H], bf16, isOutput=False)
    c1 = nc.declare_dram_parameter("c1", [1, H], f32, isOutput=False)
    c2s = nc.declare_dram_parameter("c2s", [T, H], f32, isOutput=False)
    c3 = nc.declare_dram_parameter("c3", [1, H], f32, isOutput=False)
    gagg = nc.declare_dram_parameter("gagg", [1, H], f32, isOutput=False)
    bagg = nc.declare_dram_parameter("bagg", [1, H], f32, isOutput=False)
    out = nc.declare_dram_parameter("out", [1, H], f32, isOutput=True)

    ag_in = [[nc.dram_tensor(f"agin{t}_{ph}", [EPH, H], f8)
              for ph in range(NPH)] if t > 0 else None
             for t in range(T + 1)]
    ag_out = [[nc.dram_tensor(f"agout{t}_{ph}", [E // NPH, H], f8,
                              addr_space="Shared")
               for ph in range(NPH)] if t > 0 else None
              for t in range(T + 1)]

    with tile.TileContext(nc) as tc:
        with (
            tc.tile_pool(name="singles", bufs=1) as singles,
            tc.tile_pool(name="hb", bufs=2) as hbpool,
            tc.tile_pool(name="hsh", bufs=2) as hshpool,
            tc.tile_pool(name="work", bufs=2) as work,
            # PSUM budget (8 banks of 2KB/partition, bank-granular):
            #   psm   4 x [128,512]f32 (A+B phase accumulators)
            #   psst  2 x [128,512]f32 (stats + small accumulators, rotating)
            #   psout 1 x [1,256]      (column-sum accumulator)
            tc.tile_pool(name="psm", bufs=4, space="PSUM") as psmpool,
            tc.tile_pool(name="psst", bufs=2, space="PSUM") as psstpool,
            tc.tile_pool(name="psout", bufs=1, space="PSUM") as psoutpool,
        ):
            # ---- static tiles ----
            xct_sb = singles.tile([128, EC], bf16)
            nc.sync.dma_start(xct_sb[:], XcT[:, :])
            xft_sb = singles.tile([128, E], bf16)
            for c in range(2):
                esl = slice(c * (E // 2), (c + 1) * (E // 2))
                nc.sync.dma_start(xft_sb[:, esl], XfT[:, esl])
            wi_sb = singles.tile([128, H], bf16)
            nc.sync.dma_start(wi_sb[:], Wi[:, :])
            w2_sb = singles.tile([128, T, 2, H], bf16)
            nc.sync.dma_start(
                w2_sb[:], W2s.ap().rearrange("t (kk p) n -> p t kk n", p=128))
            nft_sb = singles.tile([128, NNC], bf16)
            nc.sync.dma_start(nft_sb[:], nfT[:, :])
            wn_sb = singles.tile([128, H], bf16)
            nc.sync.dma_start(wn_sb[:], Wn[:, :])
            wm_sb = singles.tile([128, 2, H], bf16)
            nc.sync.dma_start(
                wm_sb[:], Wm.ap().rearrange("(kk p) n -> p kk n", p=128))

            # adjacency resident in SBUF (8 MB + 4 MB), k-chunked loads so
            # the first matmuls can start early
            aT_sb = singles.tile([128, KK, 2, EC], f8)
            aTr = aT.ap().rearrange("(kk two p) c -> p kk two c", two=2, p=128)
            for c in range(4):
                ksl = slice(c * (KK // 4), (c + 1) * (KK // 4))
                nc.sync.dma_start(aT_sb[:, ksl], aTr[:, ksl])
            ane_sb = singles.tile([128, KK, 2, NNC], f8)
            aner = aneT.ap().rearrange("(kk two p) c -> p kk two c",
                                       two=2, p=128)
            for c in range(2):
                ksl = slice(c * (KK // 2), (c + 1) * (KK // 2))
                nc.sync.dma_start(ane_sb[:, ksl], aner[:, ksl])

            onesH = singles.tile([128, 128], bf16)
            nc.vector.memset(onesH[:], 1.0 / H)
            onescol = singles.tile([128, 1], bf16)
            nc.vector.memset(onescol[:], 1.0)
            eps_sb = singles.tile([128, 1], f32)
            nc.vector.memset(eps_sb[:], EPS)
            h0_sb = singles.tile([128, ES, H], f32)

            def bcast_load(src_ap):
                t_ = singles.tile([128, H], f32)
                nc.sync.dma_start(t_[:], src_ap.to_broadcast([128, H]))
                return t_

            c1_bc = bcast_load(c1.ap()) if flags["c1"] else None
            c2_bc = [bcast_load(c2s.ap()[t_i:t_i + 1, :]) if flags["c2"][t_i]
                     else None for t_i in range(T)]
            c3_bc = bcast_load(c3.ap()) if flags["c3"] else None
            gagg_bc = bcast_load(gagg.ap()) if flags["gagg"] else None
            bagg_bc = bcast_load(bagg.ap()) if flags["bagg"] else None

            for _rep in range(repeat):
                _pipeline(nc, tile, mybir, _rep, nocc, rg,
                          pools=(singles, hbpool, hshpool, work, psmpool,
                                 psstpool, psoutpool),
                          tens=(out, ag_in, ag_out),
                          sbufs=(xct_sb, xft_sb, wi_sb, w2_sb, nft_sb,
                                 wn_sb, wm_sb, aT_sb, ane_sb, onesH, onescol,
                                 eps_sb, h0_sb),
                          bcs=(c1_bc, c2_bc, c3_bc, gagg_bc, bagg_bc))

    nc.compile()
    return nc


def _pipeline(nc, tile, mybir, _rep, nocc, rg, pools, tens, sbufs, bcs):
    f32 = mybir.dt.float32
    bf16 = mybir.dt.bfloat16
    f8 = mybir.dt.float8e4
    AF = mybir.ActivationFunctionType
    DR = mybir.MatmulPerfMode.DoubleRow
    (singles, hbpool, hshpool, work, psmpool, psstpool,
     psoutpool) = pools
    (out, ag_in, ag_out) = tens
    (xct_sb, xft_sb, wi_sb, w2_sb, nft_sb, wn_sb, wm_sb,
     aT_sb, ane_sb, onesH, onescol, eps_sb, h0_sb) = sbufs
    (c1_bc, c2_bc, c3_bc, gagg_bc, bagg_bc) = bcs

    # ---- gather plumbing -------------------------------------------------
    def launch_gather(t_idx, ph, hsh_tile):
        """hsh_tile [128, ES//NPH, H] fp8 -> ag_in -> AllGather."""
        nc.sync.dma_start(
            ag_in[t_idx][ph].ap().rearrange("(es p) h -> p es h", p=128),
            hsh_tile[:])
        if not nocc:
            nc.gpsimd.collective_compute(
                "AllGather", mybir.AluOpType.bypass, replica_groups=rg,
                ins=[ag_in[t_idx][ph].ap().opt()],
                outs=[ag_out[t_idx][ph].ap().opt()])

    def load_hb(t_idx, ph):
        """Load gathered h for phase ph of stage t into 2 SBUF chunks.

        Chunk c holds ranks g in [4c, 4c+4); layout [128, g4, l, two, H],
        where (g,l) maps to global kk = 4g + 2*ph + l.
        """
        chunks = []
        for c in range(2):
            hg = hbpool.tile([128, 4, 2, 2, H], f8, tag=f"hb{ph}{c}",
                             name=f"hb_{_rep}_{t_idx}_{ph}_{c}")
            if nocc:
                # timeline-sim variant (no collective support): emulate the
                # gather's local DMA traffic by reading the shard 8x.
                src = ag_in[t_idx][ph].ap().rearrange(
                    "(l two p) h -> p l two h", two=2, p=128)
                for g4 in range(4):
                    nc.sync.dma_start(hg[:, g4], src)
            else:
                src = ag_out[t_idx][ph].ap().rearrange(
                    "(g l two p) h -> p g l two h", l=2, two=2, p=128)
                nc.sync.dma_start(hg[:], src[:, c * 4:(c + 1) * 4])
            chunks.append(hg)
        return chunks

    def hb_slice(hb, kk, half):
        """Stationary [128, 2, 128] fp8 for global kk-chunk, given hb as
        {ph: [chunk0, chunk1]} of the current stage."""
        g, r = divmod(kk, 4)
        ph, l = divmod(r, 2)
        return hb[ph][g // 4][:, g % 4, l, :, half * 128:(half + 1) * 128]

    # kk consumption order: phase-0-delivered chunks first
    KK_ORDER = [4 * g + 2 * ph + l
                for ph in range(NPH) for g in range(NCORES) for l in range(2)]

    # ---- h0 = relu(X @ W_init + b_init) ----------------------------------
    # Every core computes the FULL h0 in fp8 locally (X is replicated), so
    # iteration 1 needs no collective at all. The f32 residual h0 is
    # computed separately from the local X shard.
    for es in range(ES):
        ps = psstpool.tile([128, H], f32, tag="psst",
                           name=f"psi_{_rep}_{es}")
        nc.tensor.matmul(ps[:], lhsT=xct_sb[:, es * 128:(es + 1) * 128],
                         rhs=wi_sb[:], start=True, stop=True)
        if c1_bc is not None:
            tmp = work.tile([128, H], f32, tag="tmp")
            nc.vector.tensor_add(tmp[:], ps[:], c1_bc[:])
            nc.scalar.activation(h0_sb[:, es], tmp[:], AF.Relu)
        else:
            nc.scalar.activation(h0_sb[:, es], ps[:], AF.Relu)

    hb = {ph: [hbpool.tile([128, 4, 2, 2, H], f8, tag=f"hb{ph}{c}",
                           name=f"hb_{_rep}_0_{ph}_{c}") for c in range(2)]
          for ph in range(NPH)}
    for j in range(E // 128):          # global 128-edge subtile
        g, s = divmod(j, 8)
        r, two = divmod(s, 2)
        ph, l = divmod(r, 2)
        ps = psmpool.tile([128, EPH], f32, tag="psm",
                          name=f"psf_{_rep}_{j}")
        nc.tensor.matmul(ps[:, :H], lhsT=xft_sb[:, j * 128:(j + 1) * 128],
                         rhs=wi_sb[:], start=True, stop=True)
        dst = hb[ph][g // 4][:, g % 4, l, two, :]
        if c1_bc is not None:
            tmp = work.tile([128, H], f32, tag="tmp", name=f"tpf_{_rep}_{j}")
            nc.vector.tensor_add(tmp[:], ps[:, :H], c1_bc[:])
            nc.scalar.activation(dst, tmp[:], AF.Relu)
        elif j % 2:
            nc.scalar.activation(dst, ps[:, :H], AF.Relu)
        else:
            nc.vector.tensor_scalar_max(dst, ps[:, :H], 0.0)

    # ---- T message-passing iterations ------------------------------------
    def emit_mm(t, ps_m, et, lo, hi, hb_cur):
        for kk in KK_ORDER[lo:hi]:
            for half in range(2):
                nc.tensor.matmul(
                    ps_m[half][:], lhsT=hb_slice(hb_cur, kk, half),
                    rhs=aT_sb[:, kk, :, et * EPH:(et + 1) * EPH],
                    start=(kk == KK_ORDER[0]), stop=(kk == KK_ORDER[-1]),
                    perf_mode=DR)

    def ln_stats(ps_m, width, name):
        """From psum m.T halves, compute mT (bf16), psmean, pssq."""
        mT = work.tile([128, 2, width], bf16, tag=f"mT{width}",
                       name=f"mT_{name}")
        sq = work.tile([128, 2, width], bf16, tag=f"sq{width}",
                       name=f"sq_{name}")
        for half in range(2):
            nc.vector.tensor_copy(mT[:, half], ps_m[half][:])
            nc.scalar.activation(sq[:, half], ps_m[half][:], AF.Square)
        return mT, sq

    def ln_matmuls(mT, sq, width, name):
        psmean = psstpool.tile([128, width], f32, tag="psst",
                               name=f"psmean_{name}")
        nc.tensor.matmul(psmean[:], lhsT=onesH[:], rhs=mT[:, 0],
                         start=True, stop=False)
        nc.tensor.matmul(psmean[:], lhsT=onesH[:], rhs=mT[:, 1],
                         start=False, stop=True)
        pssq = psstpool.tile([128, width], f32, tag="psst",
                             name=f"pssq_{name}")
        nc.tensor.matmul(pssq[:], lhsT=onesH[:], rhs=sq[:, 0],
                         start=True, stop=False)
        nc.tensor.matmul(pssq[:], lhsT=onesH[:], rhs=sq[:, 1],
                         start=False, stop=True)
        return psmean, pssq

    def ln_finish(mT, psmean, pssq, width, name):
        """cln = (mT - mean) * rsqrt(var + eps), bf16."""
        msq = work.tile([128, width], f32, tag=f"msq{width}",
                        name=f"msq_{name}")
        nc.scalar.activation(msq[:], psmean[:], AF.Square)
        var = work.tile([128, width], f32, tag=f"var{width}",
                        name=f"var_{name}")
        nc.vector.tensor_sub(var[:], pssq[:], msq[:])
        srt = work.tile([128, width], f32, tag=f"srt{width}",
                        name=f"srt_{name}")
        nc.scalar.activation(srt[:], var[:], AF.Sqrt, bias=eps_sb[:],
                             scale=1.0)
        rstd = work.tile([128, width], bf16, tag=f"rstd{width}",
                         name=f"rstd_{name}")
        with nc.allow_low_precision(reason="bf16 LN rstd, within err budget"):
            nc.vector.reciprocal(out=rstd[:], in_=srt[:])
        cln = work.tile([128, 2, width], bf16, tag=f"cln{width}",
                        name=f"cln_{name}")
        for half in range(2):
            nc.vector.tensor_sub(cln[:, half], mT[:, half], psmean[:])
            nc.vector.tensor_mul(cln[:, half], cln[:, half], rstd[:])
        return cln

    def update_phase(t, ph, cln, hsh_tile):
        """h = relu(h0 + cln @ W2[t]) for the 4 es-blocks of phase ph,
        write fp8 into hsh_tile, then gather."""
        for e4 in range(ES // NPH):
            es = ph * (ES // NPH) + e4
            lsl = slice(e4 * 128, (e4 + 1) * 128)
            psu = psstpool.tile([128, H], f32, tag="psst",
                               name=f"psu_{_rep}_{t}_{es}")
            nc.tensor.matmul(psu[:], lhsT=cln[:, 0, lsl], rhs=w2_sb[:, t, 0],
                             start=True, stop=False)
            nc.tensor.matmul(psu[:], lhsT=cln[:, 1, lsl], rhs=w2_sb[:, t, 1],
                             start=False, stop=True)
            tmp = work.tile([128, H], f32, tag="tmp",
                            name=f"upd_{_rep}_{t}_{es}")
            nc.vector.tensor_add(tmp[:], psu[:], h0_sb[:, es])
            if c2_bc[t] is not None:
                nc.vector.tensor_add(tmp[:], tmp[:], c2_bc[t][:])
            nc.scalar.activation(hsh_tile[:, e4], tmp[:], AF.Relu)
        launch_gather(t + 1, ph, hsh_tile)

    for t in range(T):
        ps_m = {et: [psmpool.tile([128, EPH], f32, tag="psm",
                                  name=f"psm_{_rep}_{t}_{et}_{h_}")
                     for h_ in range(2)] for et in range(NPH)}
        hsh = {ph: hshpool.tile([128, ES // NPH, H], f8, tag=f"hsh{ph}",
                                name=f"hsh_{_rep}_{t}_{ph}")
               for ph in range(NPH)}
        # ph0-sourced k-chunks of BOTH output phases first (they only
        # need AG-A of this stage and run while AG-B is in flight), then
        # phase A's ph1-sourced tail -> LN -> update -> AG-A launch ASAP;
        # phase B's tail + LN + update run under AG-A, then AG-B.
        emit_mm(t, ps_m[0], 0, 0, KK // 2, hb)
        emit_mm(t, ps_m[1], 1, 0, KK // 2, hb)
        emit_mm(t, ps_m[0], 0, KK // 2, KK, hb)
        mT_a, sq_a = ln_stats(ps_m[0], EPH, f"{_rep}_{t}_0")
        psmean_a, pssq_a = ln_matmuls(mT_a, sq_a, EPH, f"{_rep}_{t}_0")
        cln_a = ln_finish(mT_a, psmean_a, pssq_a, EPH, f"{_rep}_{t}_0")
        update_phase(t, 0, cln_a, hsh[0])
        emit_mm(t, ps_m[1], 1, KK // 2, KK, hb)
        mT_b, sq_b = ln_stats(ps_m[1], EPH, f"{_rep}_{t}_1")
        psmean_b, pssq_b = ln_matmuls(mT_b, sq_b, EPH, f"{_rep}_{t}_1")
        cln_b = ln_finish(mT_b, psmean_b, pssq_b, EPH, f"{_rep}_{t}_1")
        update_phase(t, 1, cln_b, hsh[1])
        hb = {}
        for ph in range(NPH):
            hb[ph] = load_hb(t + 1, ph)

    # ---- node aggregation: m_v.T = h.T @ adj_ne_shard.T -------------------
    ps_mv = [psmpool.tile([128, NNC], f32, tag="psm",
                          name=f"psmv_{_rep}_{h_}")
             for h_ in range(2)]
    for kk in KK_ORDER:
        for half in range(2):
            nc.tensor.matmul(
                ps_mv[half][:], lhsT=hb_slice(hb, kk, half),
                rhs=ane_sb[:, kk], start=(kk == KK_ORDER[0]),
                stop=(kk == KK_ORDER[-1]), perf_mode=DR)

    mT_v, sq_v = ln_stats(ps_mv, NNC, f"{_rep}_v")
    psmean_v, pssq_v = ln_matmuls(mT_v, sq_v, NNC, f"{_rep}_v")
    cln_v = ln_finish(mT_v, psmean_v, pssq_v, NNC, f"{_rep}_v")

    # ---- h_v = relu(nf @ Wagg[:64] + m_v_ln @ Wagg[64:] + c3);
    #      LN again; column-sum over nodes ----
    ps_out = psoutpool.tile([1, H], f32, tag="psout")
    for ns in range(NS):
        sl = slice(ns * 128, (ns + 1) * 128)
        ps_hv = psstpool.tile([128, H], f32, tag="psst",
                                name=f"pshv_{_rep}_{ns}")
        nc.tensor.matmul(ps_hv[:], lhsT=nft_sb[:, sl], rhs=wn_sb[:],
                         start=True, stop=False)
        nc.tensor.matmul(ps_hv[:], lhsT=cln_v[:, 0, sl],
                         rhs=wm_sb[:, 0], start=False, stop=False)
        nc.tensor.matmul(ps_hv[:], lhsT=cln_v[:, 1, sl],
                         rhs=wm_sb[:, 1], start=False, stop=True)
        hv = work.tile([128, H], f32, tag="hv", name=f"hv_{_rep}_{ns}")
        if c3_bc is not None:
            nc.vector.tensor_add(hv[:], ps_hv[:], c3_bc[:])
            nc.vector.tensor_scalar_max(hv[:], hv[:], 0.0)
        else:
            nc.scalar.activation(hv[:], ps_hv[:], AF.Relu)
        stats = work.tile([128, 6], f32, tag="stats")
        nc.vector.bn_stats(out=stats[:], in_=hv[:])
        mv2 = work.tile([128, 2], f32, tag="mv2")
        nc.vector.bn_aggr(out=mv2[:], in_=stats[:])
        rstd2 = work.tile([128, 1], f32, tag="rstd2")
        nc.scalar.activation(rstd2[:], mv2[:, 1:2], AF.Sqrt,
                             bias=eps_sb[:], scale=1.0)
        nc.vector.reciprocal(out=rstd2[:], in_=rstd2[:])
        ln2 = work.tile([128, H], bf16, tag="ln2", name=f"ln2_{_rep}_{ns}")
        nc.vector.tensor_scalar(
            out=ln2[:], in0=hv[:], scalar1=mv2[:, 0:1],
            scalar2=rstd2[:], op0=mybir.AluOpType.subtract,
            op1=mybir.AluOpType.mult)
        if gagg_bc is not None:
            nc.vector.tensor_mul(ln2[:], ln2[:], gagg_bc[:])
        if bagg_bc is not None:
            nc.vector.tensor_add(ln2[:], ln2[:], bagg_bc[:])
        nc.tensor.matmul(ps_out[:], lhsT=onescol[:], rhs=ln2[:],
                         start=(ns == 0), stop=(ns == NS - 1))

    out_sb = work.tile([1, H], f32, tag="osb")
    nc.vector.tensor_copy(out_sb[:], ps_out[:])
    nc.sync.dma_start(out[:, :], out_sb[:])


def prepare(inputs, repeat=1, nocc=False):
    """Host-side prep: returns (nc, in_maps) for run_bass_kernel_spmd."""
    f = {k: np.ascontiguousarray(np.asarray(v), dtype=np.float32)
         for k, v in inputs.items()}

    X = np.concatenate(
        [f["edge_aligned_node_features"], f["dir_edge_features"]], axis=1)

    # Fold LN gains into the downstream weights (host-side, exact fp32).
    g_p, b_p = f["ln_pass_g"], f["ln_pass_b"]
    g_a, b_a = f["ln_agg_g"], f["ln_agg_b"]
    W2s = (g_p[:, None] * f["W_pass"]).astype(BF)                  # [T,H,H]
    c2s = (b_p @ f["W_pass"] + f["b_pass"]).astype(np.float32)     # [T,H]
    Wm = (g_a[:, None] * f["W_agg"][NODE_D:]).astype(BF)           # [H,H]
    c3 = (b_a @ f["W_agg"][NODE_D:] + f["b_agg"]).astype(np.float32)

    Wi = np.zeros((128, H), np.float32)
    Wi[:NODE_D + EDGE_D] = f["W_init"]
    Wn = np.zeros((128, H), np.float32)
    Wn[:NODE_D] = f["W_agg"][:NODE_D]

    flags = {
        "c1": bool(np.any(f["b_init"])),
        "c2": [bool(np.any(c2s[t])) for t in range(T)],
        "c3": bool(np.any(c3)),
        "gagg": not np.all(g_a == 1.0),
        "bagg": bool(np.any(b_a)),
    }
    key = (flags["c1"], tuple(flags["c2"]), flags["c3"], flags["gagg"],
           flags["bagg"], repeat, nocc)
    if key not in _prog_cache:
        _prog_cache[key] = _build(flags, repeat=repeat, nocc=nocc)
    nc = _prog_cache[key]

    XfT = np.zeros((128, E), np.float32)
    XfT[:NODE_D + EDGE_D] = X.T
    shared = {
        "XfT": XfT.astype(BF),
        "Wi": Wi.astype(BF),
        "W2s": W2s,
        "Wn": Wn.astype(BF),
        "Wm": Wm,
        "c1": f["b_init"].reshape(1, H),
        "c2s": c2s,
        "c3": c3.reshape(1, H),
        "gagg": g_a.reshape(1, H),
        "bagg": b_a.reshape(1, H),
    }
    in_maps = []
    for c in range(NCORES):
        er = slice(c * EC, (c + 1) * EC)
        nr = slice(c * NNC, (c + 1) * NNC)
        XcT = np.zeros((128, EC), np.float32)
        XcT[:NODE_D + EDGE_D] = X[er].T
        nfT = np.zeros((128, NNC), np.float32)
        nfT[:NODE_D] = f["node_features"][nr].T
        in_maps.append(dict(
            shared,
            aT=np.ascontiguousarray(f["adj_ee"][er].T).astype(F8),
            aneT=np.ascontiguousarray(f["adj_ne"][nr].T).astype(F8),
            XcT=XcT.astype(BF),
            nfT=nfT.astype(BF),
        ))
    return nc, in_maps


def kernel(**inputs) -> np.ndarray:
    global LAST_RESULT
    from concourse.bass_utils import run_bass_kernel_spmd

    nc, in_maps = prepare(inputs)
    LAST_RESULT = run_bass_kernel_spmd(nc, in_maps, list(range(NCORES)))
    parts = [LAST_RESULT.results[c]["out"] for c in range(NCORES)]
    return np.sum(parts, axis=0, dtype=np.float32).reshape(1, H)
